# revision 65
# baseline (speedup 1.0000x reference)
"""Tensor-parallel attention block (QKV + RoPE + causal attention + out-proj)
for 8 Trainium2 NeuronCores.

Sharding: heads (16) split across 8 cores, 2 heads/core. wq/wk/wv column-
sharded, wo row-sharded; each core computes a full-shape partial output and
the host sums the 8 partials.

Layout / schedule:
  - host pre-transposes x -> xT [D, B*S]; projections are natural lhsT.T@rhs
  - q,k produced in [head_dim, seq]; scores computed TRANSPOSED ([k,q])
  - V computed directly in [seq, head_dim] layout (x-chunk stationary): no
    on-chip transposes at all
  - scores for two k-chunks go into one [128,1024] PSUM tile so a single
    Act exp covers both (Act is the attention-phase limiter)
  - softmax denominator: DVE-accumulates prob chunks + one ones-matmul per
    window; for batch-1 qw3 windows it's per-chunk PE ones-matmuls instead
    (balances DVE vs PE); 1/denom = exp(-ln(dn)) on Act (DVE reciprocal is a
    ~3.3us multi-pass lowering)
  - batch-0 attention windows and their out-proj tiles are interleaved into
    batch-1's QKV pass stream as filler segments: QKV keeps the PE dense
    while Act/DVE absorb batch-0's exp/accumulate work, and vice versa.
    Remaining out-proj is sprinkled into later windows' chunk streams so the
    PE never idles (an idle PE resets the DVFS boost 2.4->1.2GHz).
  - mask handled as multiplicative exp(mask) tiles; all-zero tiles skip
    compute, all-one tiles skip the multiply (derived from real mask values)
  - output partials written bf16 via SP-issued HWDGE DMAs (the gpsimd queue
    carries the x/weight loads); host sums partials in f32
"""

import math
import os
import sys

import numpy as np
import ml_dtypes

sys.path.insert(0, "/opt/trn_rl_repo")

import concourse.bass as bass
import concourse.mybir as mybir
from concourse.tile import TileContext
from concourse.bass_utils import run_bass_kernel_spmd

B, S, D, H = 2, 2048, 2048, 16
HD = D // H            # 128 head dim
NCORES = 8
HC = H // NCORES       # 2 heads per core
DHC = HC * HD          # 256
BS = B * S             # 4096
NDIN = D // 128        # 16 contraction chunks
W = 512                # attention q-window / matmul free size
NQW = S // W           # 4 q windows per batch
NKC = S // 128         # 16 k chunks per batch
SG = 1024              # qkv seq-group width
NSG = BS // SG         # 4
RSQRT_HD = 1.0 / math.sqrt(HD)

BF16 = mybir.dt.bfloat16
F32 = mybir.dt.float32
NPBF16 = ml_dtypes.bfloat16

SKIP, FREE, MASKED = 0, 1, 2

# stash of the last BassKernelResults for the test harness (exec_time_ns etc)
LAST_RUN = [None]
_PROGRAM_CACHE = {}


def _split_multi_waits(nc):
    """Walrus codegen allows only 1 embedded sync-wait per instruction (2 for
    EventSemaphore). Tile's sem-assignment can emit more; hoist the excess into
    standalone InstEventSemaphore waits on the same engine, just before."""
    n = 0
    for blk in nc.m.functions[0].blocks:
        out = []
        for inst in blk.instructions:
            si = getattr(inst, "sync_info", None)
            cap = 2 if isinstance(inst, mybir.InstEventSemaphore) else 1
            if si is not None and si.on_wait and len(si.on_wait) > cap:
                waits = list(si.on_wait)
                for w in waits[:-cap]:
                    n += 1
                    ev = mybir.InstEventSemaphore(
                        name=f"{inst.name}_xw{n}",
                        ins=[], outs=[],
                        sync_info=mybir.SyncInfo(on_wait=[w], on_update=[]))
                    ev.engine = inst.engine
                    out.append(ev)
                si.on_wait = waits[-cap:]
            out.append(inst)
        blk.instructions = out


def _build(cls_key):
    """Build the per-core Bass program. cls_key: tuple[NQW][NKC] of SKIP/FREE/MASKED."""
    cls = [list(row) for row in cls_key]
    nc = bass.Bass()

    xT = nc.declare_dram_parameter("xT", [D, BS], BF16, isOutput=False)
    wqT = nc.declare_dram_parameter("wqT", [D, DHC], BF16, isOutput=False)
    wkT = nc.declare_dram_parameter("wkT", [D, DHC], BF16, isOutput=False)
    wvT = nc.declare_dram_parameter("wvT", [D, DHC], BF16, isOutput=False)
    woT = nc.declare_dram_parameter("woT", [DHC, D], BF16, isOutput=False)
    trig = nc.declare_dram_parameter("trig", [128, 2 * S], BF16, isOutput=False)
    emaskT = nc.declare_dram_parameter("emaskT", [S, S], BF16, isOutput=False)
    out_d = nc.declare_dram_parameter("out", [BS, D], BF16, isOutput=True)

    with TileContext(nc) as tc:
        with (
            tc.tile_pool(name="consts", bufs=1) as consts,
            tc.tile_pool(name="xt", bufs=13) as xtp,
            tc.tile_pool(name="rsw", bufs=2) as rswp,
            tc.tile_pool(name="rm", bufs=2) as rmp,
            tc.tile_pool(name="probs", bufs=5) as prp,
            tc.tile_pool(name="acc", bufs=2) as accp,
            tc.tile_pool(name="emask", bufs=12) as emp,
            tc.tile_pool(name="small", bufs=2) as smp,
            tc.tile_pool(name="lnt", bufs=1) as lnp,
            tc.tile_pool(name="outsb", bufs=6) as outp,
            tc.tile_pool(name="psA", bufs=2, space="PSUM") as psA,
            tc.tile_pool(name="psB", bufs=2, space="PSUM") as psB,
            tc.tile_pool(name="psC", bufs=2, space="PSUM") as psC,
        ):
            # persistent tiles
            q_sb = [consts.tile([128, BS], BF16, tag=f"q{h}", name=f"q{h}") for h in range(HC)]
            k_sb = [consts.tile([128, BS], BF16, tag=f"k{h}", name=f"k{h}") for h in range(HC)]
            a_sb = [consts.tile([128, BS], BF16, tag=f"a{h}", name=f"a{h}") for h in range(HC)]
            vT_sb = consts.tile([128, B * NKC * DHC], BF16, tag="vT", name="vT")
            ones = consts.tile([128, 128], BF16, tag="ones", name="ones")
            wdum = consts.tile([128, 128], BF16, tag="wdum", name="wdum")
            nc.gpsimd.memset(wdum, 1.0)
            nc.vector.memset(ones, 1.0)
            w_all = [consts.tile([128, NDIN, DHC], BF16, tag=f"wall{i}", name=f"wall{i}")
                     for i in range(3)]
            trig_sb = consts.tile([128, 2 * S], BF16, tag="trig", name="trig")
            woT_sb = [consts.tile([128, D], BF16, tag=f"wo{h}", name=f"wo{h}")
                      for h in range(HC)]

            # warm the PE clock (HAM releases the 1.2GHz throttle after ~3.4us
            # of sustained activity) while the first DMAs are in flight; uses
            # a gpsimd-memset dummy so it doesn't wait on the DVE coming up
            wu = psB.tile([128, 2 * W], F32, tag="psB", name="warmup")
            for i in range(90):
                nc.tensor.matmul(wu[:, 0:128], lhsT=wdum, rhs=wdum, start=True, stop=True)

            # ---- em mask prefetch ----
            em_tiles = {}         # (b, qw) -> {c: tile}

            def prefetch_em(b, qw):
                if b >= B or qw >= NQW or (b, qw) in em_tiles:
                    return
                tiles = {}
                for c in range(NKC):
                    if cls[qw][c][0] == MASKED:
                        em = emp.tile([128, W], BF16, tag="em", name=f"em{b}_{qw}_{c}")
                        nc.gpsimd.dma_start(
                            out=em,
                            in_=emaskT[c * 128:(c + 1) * 128, qw * W:(qw + 1) * W])
                        tiles[c] = em
                em_tiles[(b, qw)] = tiles

            # ---- out-projection units ----
            pending = []          # (st, dgg) out-proj units not yet emitted
            copy_ctr = [0]
            final_phase = [False]  # True once no more x/em loads need gpsimd

            def emit_unit(drain=False):
                st, dgg = pending.pop(0)
                ops = [psA.tile([128, W], F32, tag="psA", name=f"o{st}_{dgg}_{d2}")
                       for d2 in range(2)]
                for h in range(HC):
                    for d2 in range(2):
                        dg = dgg * 2 + d2
                        nc.tensor.matmul(
                            ops[d2], lhsT=a_sb[h][:, st * 128:(st + 1) * 128],
                            rhs=woT_sb[h][:, dg * W:(dg + 1) * W],
                            start=(h == 0), stop=(h == HC - 1))
                for d2 in range(2):
                    dg = dgg * 2 + d2
                    ob = outp.tile([128, W], BF16, tag="ob", name=f"ob{st}_{dg}")
                    copy_ctr[0] += 1
                    # When Act is under exp pressure it gets 1/3 of the
                    # copies; during drains split evenly (PE-bound drain)
                    mod = 2 if drain else 3
                    if copy_ctr[0] % mod == 0:
                        nc.scalar.copy(ob, ops[d2])
                    else:
                        nc.vector.tensor_copy(ob, ops[d2])
                    # alternate issue queues in the final drain: one SP
                    # sequencer alone caps the drain at 2x565ns per unit,
                    # below the PE's 852ns pace. Earlier, keep SP only: the
                    # gpsimd queue carries x/em loads that must not queue
                    # behind copy-gated out-DMAs.
                    eng = nc.gpsimd if (final_phase[0] and d2 == 1) else nc.sync
                    eng.dma_start(
                        out=out_d[st * 128:(st + 1) * 128, dg * W:(dg + 1) * W], in_=ob)

            # ---- one attention window (both heads, h-inner pipelining) ----
            def emit_window(b, qw, sprinkle):
                active = [c for c in range(NKC) if cls[qw][c][0] != SKIP]
                if not active:
                    return
                ems = em_tiles[(b, qw)]
                qc = b * S + qw * W
                nact = len(active)
                atts, accs = {}, {}
                for h in range(HC):
                    atts[h] = psC.tile([128, W], F32, tag="psC", name=f"att{b}_{h}_{qw}")
                    accs[h] = accp.tile([128, W], BF16, tag="acc", name=f"acc{b}_{h}_{qw}")

                rcs = {}

                def emit_tail_act(h):
                    # per-head softmax tail, Act part only: window denominator
                    # matmul, then 1/dn as exp(-ln(dn)) on Act (Ln and Exp
                    # share one activation table so there's no reload). The
                    # DVE normalize is deferred until both heads' chunks are
                    # done — emitting it here would head-of-line block the
                    # in-order DVE queue (h1's mask/acc ops) on the Act chain.
                    dn = psA.tile([128, W], F32, tag="psA", name=f"dnl{b}_{h}_{qw}")
                    nc.tensor.matmul(dn, lhsT=ones, rhs=accs[h], start=True, stop=True)
                    lt = lnp.tile([128, W], F32, tag="lnt", name=f"ln{b}_{h}_{qw}")
                    rc = smp.tile([128, W], F32, tag="recip", name=f"rc{b}_{h}_{qw}")
                    with tc.high_priority():
                        nc.scalar.activation(lt, dn,
                                             mybir.ActivationFunctionType.Ln)
                        nc.scalar.activation(rc, lt,
                                             mybir.ActivationFunctionType.Exp,
                                             scale=-1.0)
                    rcs[h] = rc

                for pi in range(0, nact, 2):
                    pair = active[pi:pi + 2]
                    last_pair = pi + 2 >= nact
                    for h in range(HC):
                        sp = psB.tile([128, 2 * W], F32, tag="psB",
                                      name=f"sc{b}_{h}_{qw}_{pi}")
                        for j, c in enumerate(pair):
                            kc = b * S + c * 128
                            nc.tensor.matmul(sp[:, j * W:(j + 1) * W],
                                             lhsT=k_sb[h][:, kc:kc + 128],
                                             rhs=q_sb[h][:, qc:qc + W],
                                             start=True, stop=True)
                        pb = prp.tile([128, 2 * W], BF16, tag="probs",
                                      name=f"pb{b}_{h}_{qw}_{pi}")
                        nc.scalar.activation(pb, sp,
                                             mybir.ActivationFunctionType.Exp,
                                             scale=RSQRT_HD)
                        for j, c in enumerate(pair):
                            ci = pi + j
                            pbj = pb[:, j * W:(j + 1) * W]
                            if cls[qw][c][0] == MASKED:
                                nc.vector.tensor_mul(pbj, pbj, ems[c])
                            if ci == 0:
                                nc.vector.tensor_copy(accs[h], pbj)
                            else:
                                nc.vector.tensor_add(accs[h], accs[h], pbj)
                            g = b * NKC + c
                            o0 = g * DHC + h * 128
                            nc.tensor.matmul(atts[h], lhsT=vT_sb[:, o0:o0 + 128],
                                             rhs=pbj,
                                             start=(ci == 0), stop=(ci == nact - 1))
                            # out-proj sprinkling: only from ci>=2 (earlier
                            # slots would head-of-line block the PE on the
                            # previous window's not-yet-done normalize)
                            if (sprinkle and pending and ci >= 2
                                    and (ci % 2 == 0 or nact <= 8)):
                                emit_unit()
                        # each head's Act recip chain starts as soon as that
                        # head's chunks are done: h0's overlaps h1's last
                        # pair, h1's overlaps the next window's start
                        if last_pair:
                            emit_tail_act(h)
                # deferred normalizes: by now each rc is (nearly) ready, so
                # these run back-to-back and release the att PSUM tiles fast
                for h in range(HC):
                    with tc.high_priority():
                        nc.vector.tensor_mul(a_sb[h][:, qc:qc + W], atts[h], rcs[h])
                for st_local in range(W // 128):
                    st = b * NKC + qw * (W // 128) + st_local
                    for dgg in range(2):
                        pending.append((st, dgg))

            # attention filler segments, pumped between QKV passes: batch-0
            # windows go into sg2/sg3; once rope(b1) is emitted (mid-sg3),
            # the first batch-1 windows join the queue too.
            filler = [(0, qw) for qw in range(NQW)]

            def pump_filler():
                if filler:
                    b, qw = filler.pop(0)
                    emit_window(b, qw, sprinkle=(b == 1))
                else:
                    for _ in range(2):
                        if pending:
                            emit_unit(drain=True)

            # ---- startup DMAs, ordered by first use: the q/dh0 pass needs
            # all of wq plus the x-tiles in di order ----
            nc.gpsimd.dma_start(
                out=w_all[0][:, 0:8, :],
                in_=wqT[0:1024, :].rearrange("(n p) m -> p n m", p=128))
            sg0_tiles = []

            def sg0_xt(dj):
                tb = xtp.tile([128, 2, SG], BF16, tag="xt", name=f"xt0_{dj}")
                nc.gpsimd.dma_start(
                    out=tb,
                    in_=xT[dj * 256:(dj + 1) * 256, 0:SG].rearrange("(n p) m -> p n m", p=128))
                sg0_tiles.append(tb)

            for dj in range(2):
                sg0_xt(dj)
            nc.gpsimd.dma_start(
                out=w_all[0][:, 8:16, :],
                in_=wqT[1024:2048, :].rearrange("(n p) m -> p n m", p=128))
            for dj in range(2, 8):
                sg0_xt(dj)
            nc.gpsimd.dma_start(
                out=w_all[1], in_=wkT.rearrange("(n p) m -> p n m", p=128))
            nc.gpsimd.dma_start(
                out=w_all[2], in_=wvT.rearrange("(n p) m -> p n m", p=128))

            # ---- QKV projections (with batch-0 attention filled into the
            # batch-1 seq-groups) ----
            for sg in range(NSG):
                interleave = sg >= 2
                xts = []
                for dj in range(8):
                    if sg == 0:
                        tb = sg0_tiles[dj]
                    else:
                        tb = xtp.tile([128, 2, SG], BF16, tag="xt", name=f"xt{sg}_{dj}")
                        nc.gpsimd.dma_start(
                            out=tb,
                            in_=xT[dj * 256:(dj + 1) * 256,
                                   sg * SG:(sg + 1) * SG].rearrange("(n p) m -> p n m", p=128))
                    for k2 in range(2):
                        xts.append(tb[:, k2, :])
                if sg == 1:
                    # needed only from rope / out-proj onward; keep out of the
                    # startup critical path but ahead of later bulk
                    nc.gpsimd.dma_start(out=trig_sb, in_=trig[:, :])
                    for h in range(HC):
                        nc.gpsimd.dma_start(out=woT_sb[h], in_=woT[h * 128:(h + 1) * 128, :])
                    prefetch_em(0, 0)
                    prefetch_em(0, 1)
                if sg == 2:
                    prefetch_em(0, 2)
                    prefetch_em(0, 3)
                if sg == 3:
                    prefetch_em(1, 0)
                    prefetch_em(1, 1)
                # q, k: weight-stationary, [head_dim, seq] output
                for ti in range(2):
                    for dh in range(2):
                        ps = [psA.tile([128, W], F32, tag="psA", name=f"psA{sg}_{ti}_{dh}_{wi}")
                              for wi in range(2)]
                        for di in range(NDIN):
                            for wi in range(2):
                                nc.tensor.matmul(
                                    ps[wi], lhsT=w_all[ti][:, di, dh * 128:(dh + 1) * 128],
                                    rhs=xts[di][:, wi * W:(wi + 1) * W],
                                    start=(di == 0), stop=(di == NDIN - 1))
                        for wi in range(2):
                            c0 = sg * SG + wi * W
                            dst = (q_sb if ti == 0 else k_sb)[dh]
                            # DVE, not Act: keeps Act free for attention exp
                            with tc.high_priority():
                                nc.vector.tensor_copy(dst[:, c0:c0 + W], ps[wi])
                        if interleave:
                            pump_filler()
                # rope for batch b once its q,k are projected (before the v
                # section so rope(b1) overlaps sg3's v matmuls and the first
                # b1 windows can be pumped into the v stream right after)
                if sg % 2 == 1:
                    b = sg // 2
                    HS = S // 2
                    for tens in (q_sb, k_sb):
                        for h in range(HC):
                            src = tens[h]
                            for hs in range(2):
                                c2 = b * S + hs * HS
                                t2 = hs * HS
                                sw = rswp.tile([128, HS], BF16, tag="rsw",
                                               name=f"sw{sg}_{h}_{hs}")
                                # SP-issued HWDGE: these wait on the q/k
                                # copies and would head-of-line block the
                                # gpsimd queue
                                nc.sync.dma_start(out=sw[0:64, :],
                                                  in_=src[64:128, c2:c2 + HS])
                                nc.sync.dma_start(out=sw[64:128, :],
                                                  in_=src[0:64, c2:c2 + HS])
                                mcc = rmp.tile([128, HS], BF16, tag="mcc",
                                               name=f"mcc{sg}_{h}_{hs}")
                                mss = rmp.tile([128, HS], BF16, tag="mss",
                                               name=f"mss{sg}_{h}_{hs}")
                                nc.vector.tensor_mul(mcc, src[:, c2:c2 + HS],
                                                     trig_sb[:, t2:t2 + HS])
                                nc.vector.tensor_mul(mss, sw,
                                                     trig_sb[:, S + t2:S + t2 + HS])
                                nc.vector.tensor_add(src[:, c2:c2 + HS], mcc, mss)
                # v: x-chunk stationary, direct [seq, head_dim] output
                for sc in range(SG // 128):
                    vps = psB.tile([128, 2 * W], F32, tag="psB", name=f"vps{sg}_{sc}")
                    for di in range(NDIN):
                        nc.tensor.matmul(
                            vps[:, 0:DHC], lhsT=xts[di][:, sc * 128:(sc + 1) * 128],
                            rhs=w_all[2][:, di, :],
                            start=(di == 0), stop=(di == NDIN - 1))
                    g = sg * (SG // 128) + sc     # global 128-token chunk id
                    with tc.high_priority():
                        nc.vector.tensor_copy(vT_sb[:, g * DHC:(g + 1) * DHC], vps[:, 0:DHC])
                    if interleave and sc % 2 == 1:
                        pump_filler()

            # leftover batch-0 work not absorbed by the QKV stream
            while filler:
                pump_filler()

            # ---- batch-1 attention (+ all remaining out-projection) ----
            for qw in range(NQW):
                if qw + 1 < NQW:
                    prefetch_em(1, qw + 1)
                else:
                    final_phase[0] = True
                emit_window(1, qw, sprinkle=True)
            while pending:
                emit_unit(drain=True)
    _split_multi_waits(nc)
    return nc


def _prepare(x, freqs_cos, freqs_sin, mask, wq, wk, wv, wo):
    x = np.asarray(x, dtype=np.float32)
    wq = np.asarray(wq, dtype=np.float32)
    wk = np.asarray(wk, dtype=np.float32)
    wv = np.asarray(wv, dtype=np.float32)
    wo = np.asarray(wo, dtype=np.float32)
    fc = np.asarray(freqs_cos, dtype=np.float32)
    fs = np.asarray(freqs_sin, dtype=np.float32)
    mask = np.asarray(mask, dtype=np.float32)

    xT = np.ascontiguousarray(x.reshape(BS, D).T).astype(NPBF16)

    cosT = fc.T                      # [64, S]
    sinT = fs.T
    cos_dup = np.vstack([cosT, cosT])
    sin_sgn = np.vstack([-sinT, sinT])
    trig = np.ascontiguousarray(np.hstack([cos_dup, sin_sgn])).astype(NPBF16)

    em = np.exp(mask).T              # [k, q]; exp(-inf)=0, exp(0)=1
    emaskT = np.ascontiguousarray(em).astype(NPBF16)
    cls = []
    for qw in range(NQW):
        row = []
        for c in range(NKC):
            t = emaskT[c * 128:(c + 1) * 128, qw * W:(qw + 1) * W]
            if not t.any():
                row.append((SKIP, 0))
            elif (t == NPBF16(1.0)).all():
                row.append((FREE, 0))
            else:
                row.append((MASKED, 0))
        cls.append(tuple(row))
    cls_key = tuple(cls)

    # deinterleave perm: even dims then odd dims, per head
    ridx = np.concatenate([np.arange(0, HD, 2), np.arange(1, HD, 2)])
    in_maps = []
    for core in range(NCORES):
        heads = [core * HC + h for h in range(HC)]
        qk_rows = np.concatenate([g * HD + ridx for g in heads])
        v_rows = np.concatenate([np.arange(g * HD, (g + 1) * HD) for g in heads])
        m = {
            "xT": xT,
            "wqT": np.ascontiguousarray(wq[qk_rows].T).astype(NPBF16),
            "wkT": np.ascontiguousarray(wk[qk_rows].T).astype(NPBF16),
            "wvT": np.ascontiguousarray(wv[v_rows].T).astype(NPBF16),
            "woT": np.ascontiguousarray(wo[:, v_rows].T).astype(NPBF16),
            "trig": trig,
            "emaskT": emaskT,
        }
        in_maps.append(m)
    return in_maps, cls_key


def kernel(x, start_pos, freqs_cos, freqs_sin, mask, wq, wk, wv, wo):
    in_maps, cls_key = _prepare(x, freqs_cos, freqs_sin, mask, wq, wk, wv, wo)
    nc = _PROGRAM_CACHE.get(cls_key)
    if nc is None:
        nc = _build(cls_key)
        _PROGRAM_CACHE[cls_key] = nc
    res = run_bass_kernel_spmd(
        nc, in_maps, list(range(NCORES)),
        trace=bool(os.environ.get("KERNEL_TRACE")),
        tmpdir=os.environ.get("KERNEL_TRACE_DIR") or None)
    LAST_RUN[0] = res
    out = np.zeros([BS, D], np.float32)
    for r in res.results:
        out += np.asarray(r["out"], dtype=np.float32)
    return out.reshape(B, S, D)
